# revision 1
# baseline (speedup 1.0000x reference)
"""Trainium2 Bass kernel for nn_CharacterMaskAttentionHead.

Sharding: 8 cores = 4 images x 2 H-halves (data-parallel over batch, spatial
split within each image). Bottom halves are H-flipped on the host so every core
runs the identical SPMD program on a "top slab" (true image edge at the top,
4-row interior halo at the bottom). GroupNorm statistics are exact: each core
reduces its half, per-pair partial sums are combined with a tiny (32x2 float)
in-kernel AllReduce that overlaps with kernel-branch compute.

Per core:
  - mask tower: 4x (3x3 conv -> GN(32) -> ReLU) on a padded [86 x 162] slab,
    conv as 9 shifted matmuls accumulating in PSUM (float32r, full PE rate).
  - kernel tower: replicated within each pair on the host-resized 40x40 grid
    (GN is local there).
  - top-k gather as PE transpose + one-hot selection matmul (host computes
    jax-exact top-k indices, baked into a per-core selection-matrix input).
  - kow 1x1 as 25 matmuls; dynamic-conv einsum as 4x27 matmuls -> DMA out.
Host: pads/flips/rounds inputs, applies the jax-exact bilinear-antialias
resize (0.7% of total FLOPs, memory-bound), computes top-k, reassembles.
"""
import sys
import contextlib

sys.path.insert(0, '/opt/trn_rl_repo')

import numpy as np

import concourse.bacc as bacc
import concourse.bass as bass
import concourse.tile as tile
from concourse import mybir
from concourse.bass_utils import run_bass_kernel_spmd
from concourse.masks import make_identity

F32 = mybir.dt.float32
F32R = mybir.dt.float32r

B, CIN, H, W = 4, 256, 160, 160
C, L, S, KSEL = 128, 25, 40, 16
CINP = CIN + 2
HALF = 80                 # output rows per core
SLAB = 84                 # conv rows per layer (80 + 4 halo)
ROWS, COLS = 86, 162      # padded slab buffer
GRID_P = S + 2            # padded 40x40 grid (42)
NT = 28                   # mask conv tiles (3 rows x 160)
KL = KSEL * L             # 400
NPOS = S * S              # 1600
NCH = 13                  # ceil(1600/128)
EPS = 1e-5

_PROGRAM_CACHE = {}


def _round_f32r(x):
    """Round fp32 to the f32r grid (12 mantissa bits) so DMA-fed float32r
    tiles hold values the PE will not re-round differently in sim vs HW."""
    b = np.ascontiguousarray(x, np.float32).view(np.uint32)
    b = ((b + 0x800) & np.uint32(0xFFFFF000)).astype(np.uint32)
    return b.view(np.float32)


def resize_weight_mat(in_size, out_size):
    """jax.image.resize bilinear+antialias weight matrix; out = Wmat @ in."""
    inv_scale = in_size / out_size
    kernel_scale = max(inv_scale, 1.0)
    sample_f = (np.arange(out_size) + 0.5) * inv_scale - 0.5
    x = np.abs(sample_f[:, None] - np.arange(in_size)[None, :]) / kernel_scale
    w = np.maximum(0.0, 1.0 - x)
    tot = w.sum(axis=1, keepdims=True)
    w = np.where(np.abs(tot) > 1000 * np.finfo(np.float32).eps, w / tot, 0.0)
    valid = (sample_f >= -0.5) & (sample_f <= in_size - 0.5)
    return np.where(valid[:, None], w, 0.0)


def topk_idx(vals, k):
    """jax.lax.top_k indices: descending value, ties -> lower index first."""
    return np.argsort(-vals, axis=-1, kind='stable')[..., :k]


# --------------------------------------------------------------------------
# device program
# --------------------------------------------------------------------------

def build_program(use_f32r=True, use_cc=True):
    # use_cc: True (all), False (none), or set of stat-stage indices 0..4
    cc_set = (set(range(5)) if use_cc is True
              else (set() if use_cc is False else set(use_cc)))
    key = (use_f32r, tuple(sorted(cc_set)))
    if key in _PROGRAM_CACHE:
        return _PROGRAM_CACHE[key]
    ADT = F32R if use_f32r else F32

    nc = bacc.Bacc("TRN2", target_bir_lowering=False, num_devices=8)

    img = nc.dram_tensor("img", (CINP, ROWS, COLS), ADT, kind="ExternalInput")
    kin = nc.dram_tensor("kin", (CINP, GRID_P, GRID_P), ADT, kind="ExternalInput")
    wdr = {}
    for pre in ("f", "k"):
        wdr[f"{pre}w0a"] = nc.dram_tensor(f"{pre}w0a", (128, 9, 128), ADT, kind="ExternalInput")
        wdr[f"{pre}w0b"] = nc.dram_tensor(f"{pre}w0b", (128, 9, 128), ADT, kind="ExternalInput")
        wdr[f"{pre}w0c"] = nc.dram_tensor(f"{pre}w0c", (2, 9, 128), ADT, kind="ExternalInput")
        for l in (1, 2, 3):
            wdr[f"{pre}w{l}"] = nc.dram_tensor(f"{pre}w{l}", (128, 9, 128), ADT, kind="ExternalInput")
    fowT_d = nc.dram_tensor("fowT", (128, 128), ADT, kind="ExternalInput")
    kowT_d = nc.dram_tensor("kowT", (128, C * L), F32, kind="ExternalInput")
    mask_gb_d = nc.dram_tensor("mask_gb", (128, 10), F32, kind="ExternalInput")
    k_gb_d = nc.dram_tensor("k_gb", (128, 8), F32, kind="ExternalInput")
    kobT_d = nc.dram_tensor("kobT", (128, L), F32, kind="ExternalInput")
    onehot_g_d = nc.dram_tensor("onehot_g", (128, 32), F32, kind="ExternalInput")
    onehot_b_d = nc.dram_tensor("onehot_b", (32, 128), F32, kind="ExternalInput")
    selP_d = nc.dram_tensor("selP", (128, NCH * KSEL), F32, kind="ExternalInput")
    out_d = nc.dram_tensor("out", (KL, HALF * W), F32, kind="ExternalOutput")
    cc_d = [(nc.dram_tensor(f"ccin{l}", (32, 2), F32),
             nc.dram_tensor(f"ccout{l}", (32, 2), F32)) for l in range(5)]

    with tile.TileContext(nc) as tc, contextlib.ExitStack() as ctx:
        consts = ctx.enter_context(tc.tile_pool(name="consts", bufs=1))
        acts = ctx.enter_context(tc.tile_pool(name="acts", bufs=1))
        wpool = ctx.enter_context(tc.tile_pool(name="wpool", bufs=3))
        small = ctx.enter_context(tc.tile_pool(name="small", bufs=1))
        stage = ctx.enter_context(tc.tile_pool(name="stage", bufs=2))
        kpool = ctx.enter_context(tc.tile_pool(name="kpool", bufs=1))
        ps_conv = ctx.enter_context(tc.tile_pool(name="ps_conv", bufs=4, space="PSUM"))
        ps_k = ctx.enter_context(tc.tile_pool(name="ps_k", bufs=2, space="PSUM"))
        ps_small = ctx.enter_context(tc.tile_pool(name="ps_small", bufs=2, space="PSUM"))

        # ------------- constants -------------
        def load_const(dram, shape, dt=F32):
            t = consts.tile(shape, dt, tag=dram.name, name=dram.name)
            nc.sync.dma_start(out=t, in_=dram[tuple(slice(None) for _ in shape)])
            return t

        onehot_g = load_const(onehot_g_d, [128, 32])
        onehot_b = load_const(onehot_b_d, [32, 128])
        mask_gb = load_const(mask_gb_d, [128, 10])
        k_gb = load_const(k_gb_d, [128, 8])
        kobT = load_const(kobT_d, [128, L])
        selP = load_const(selP_d, [128, NCH * KSEL])
        fowT = load_const(fowT_d, [128, 128], ADT)
        eps32 = consts.tile([32, 1], F32, tag="eps32", name="eps32")
        nc.vector.memset(eps32, EPS)
        ident = consts.tile([128, 128], F32, tag="ident", name="ident")
        make_identity(nc, ident)

        # activation slabs (ping-pong); pads zeroed via tensor_copy from an
        # F32 zero tile (memset cannot produce f32r, tensor_copy can)
        zsrc = consts.tile([128, COLS], F32, tag="zsrc", name="zsrc")
        nc.vector.memset(zsrc, 0.0)
        zcol = zsrc[:, 0:ROWS].rearrange("p (a b) -> p a b", b=1)

        X = [acts.tile([128, ROWS, COLS], ADT, tag="X0", name="X0"),
             acts.tile([128, ROWS, COLS], ADT, tag="X1", name="X1")]
        for x in X:
            nc.vector.tensor_copy(x[:, 0, :], zsrc[:, :])
            nc.vector.tensor_copy(x[:, ROWS - 1, :], zsrc[:, :])
            nc.vector.tensor_copy(x[:, :, 0:1], zcol)
            nc.vector.tensor_copy(x[:, :, COLS - 1:COLS], zcol)
        KB = [kpool.tile([128, GRID_P, GRID_P], ADT, tag="KB0", name="KB0"),
              kpool.tile([128, GRID_P, GRID_P], ADT, tag="KB1", name="KB1")]
        zcolk = zsrc[:, 0:GRID_P].rearrange("p (a b) -> p a b", b=1)
        for x in KB:
            nc.vector.tensor_copy(x[:, 0, :], zsrc[:, 0:GRID_P])
            nc.vector.tensor_copy(x[:, GRID_P - 1, :], zsrc[:, 0:GRID_P])
            nc.vector.tensor_copy(x[:, :, 0:1], zcolk)
            nc.vector.tensor_copy(x[:, :, GRID_P - 1:GRID_P], zcolk)

        # conv weights: shared-tag lazy rotation within each tower
        def load_w(nm, tag):
            dr = wdr[nm]
            t = wpool.tile([dr.shape[0], 9, 128], ADT, tag=tag, name=nm)
            nc.sync.dma_start(out=t, in_=dr[:, :, :])
            return t

        # kin chunks (tags shared with later k-branch tiles to save SBUF)
        kinA = kpool.tile([128, GRID_P, GRID_P], ADT, tag="sh_kow", name="sh_kow")
        kinB = kpool.tile([128, GRID_P, GRID_P], ADT, tag="sh_hidT", name="sh_hidT")
        kinC = kpool.tile([2, GRID_P, GRID_P], ADT, tag="sh_c", name="sh_c")
        nc.sync.dma_start(out=kinA, in_=kin[0:128, :, :])
        nc.sync.dma_start(out=kinB, in_=kin[128:256, :, :])
        nc.sync.dma_start(out=kinC, in_=kin[256:258, :, :])

        # ------------- GroupNorm helper -------------
        def emit_gn(buf, nrows_valid, ncols, divisor, gb_ap, cc, fam):
            """stats over buf rows [1, 1+nrows_valid), cols [1, 1+ncols);
            returns [128,2] (a|b) tile for the apply pass."""
            stats = small.tile([128, nrows_valid, 6], F32, tag=f"bnst{fam}", name=f"bnst{fam}")
            for r in range(nrows_valid):
                nc.vector.bn_stats(
                    out=stats[:, r, :],
                    in_=buf[:, 1 + r, 1:1 + ncols].bitcast(F32))
            mv = small.tile([128, 2], F32, tag=f"mv{fam}", name=f"mv{fam}")
            nc.vector.bn_aggr(out=mv[:], in_=stats.rearrange("p a b -> p (a b)"))

            sq = small.tile([128, 2], F32, tag=f"sq{fam}", name=f"sq{fam}")
            nc.vector.tensor_mul(sq[:, 1:2], mv[:, 0:1], mv[:, 0:1])
            nc.vector.tensor_add(sq[:, 1:2], sq[:, 1:2], mv[:, 1:2])
            nc.vector.tensor_copy(sq[:, 0:1], mv[:, 0:1])

            pg = ps_small.tile([32, 2], F32, tag="pss", name="pss")
            nc.tensor.matmul(pg, onehot_g[:], sq[:], start=True, stop=True)
            g32 = small.tile([32, 2], F32, tag=f"g32{fam}", name=f"g32{fam}")
            nc.vector.tensor_copy(g32[:], pg)

            if cc is not None:
                ccin_d, ccout_d = cc
                nc.sync.dma_start(out=ccin_d[:, :], in_=g32[:])
                nc.gpsimd.collective_compute(
                    "AllReduce", mybir.AluOpType.add,
                    replica_groups=[[0, 1], [2, 3], [4, 5], [6, 7]],
                    ins=[ccin_d[:, :]], outs=[ccout_d[:, :]])
                g32 = small.tile([32, 2], F32, tag=f"g32r{fam}", name=f"g32r{fam}")
                nc.sync.dma_start(out=g32[:], in_=ccout_d[:, :])

            nc.vector.tensor_scalar_mul(g32[:], g32[:], 1.0 / divisor)
            msq = small.tile([32, 1], F32, tag=f"msq{fam}", name=f"msq{fam}")
            nc.vector.tensor_mul(msq[:], g32[:, 0:1], g32[:, 0:1])
            nc.vector.tensor_sub(g32[:, 1:2], g32[:, 1:2], msq[:])
            nc.scalar.activation(out=g32[:, 1:2], in_=g32[:, 1:2],
                                 func=mybir.ActivationFunctionType.Sqrt,
                                 bias=eps32[:], scale=1.0)
            nc.vector.reciprocal(g32[:, 1:2], g32[:, 1:2])

            pb = ps_small.tile([128, 2], F32, tag="pss", name="pss")
            nc.tensor.matmul(pb, onehot_b[:], g32[:], start=True, stop=True)
            mr = small.tile([128, 2], F32, tag=f"mr{fam}", name=f"mr{fam}")
            nc.vector.tensor_copy(mr[:], pb)

            ab = small.tile([128, 2], F32, tag=f"ab{fam}", name=f"ab{fam}")
            nc.vector.tensor_mul(ab[:, 0:1], gb_ap[:, 0:1], mr[:, 1:2])
            nc.vector.tensor_mul(ab[:, 1:2], mr[:, 0:1], ab[:, 0:1])
            nc.vector.tensor_sub(ab[:, 1:2], gb_ap[:, 1:2], ab[:, 1:2])
            return ab

        def apply_gn(dst, ab, nrows, ncols, chunk=None):
            chunk = chunk or nrows
            r = 0
            while r < nrows:
                take = min(chunk, nrows - r)
                nc.scalar.activation(
                    out=dst[:, 1 + r:1 + r + take, 1:1 + ncols],
                    in_=dst[:, 1 + r:1 + r + take, 1:1 + ncols].bitcast(F32),
                    func=mybir.ActivationFunctionType.Relu,
                    scale=ab[:, 0:1], bias=ab[:, 1:2])
                r += take

        # ------------- conv emitters -------------
        def mask_conv_layer(src_fn, wts, dst):
            n_chunks = len(wts)
            for t in range(NT):
                ps = ps_conv.tile([128, 3, W], F32, tag="pc", name="pc")
                first = True
                for ci in range(n_chunks):
                    rhs_tile = src_fn(t, ci)
                    for tap in range(9):
                        dy, dx = divmod(tap, 3)
                        nc.tensor.matmul(
                            ps, wts[ci][:, tap, :],
                            rhs_tile[:, dy:dy + 3, dx:dx + W],
                            start=first,
                            stop=(ci == n_chunks - 1 and tap == 8))
                        first = False
                nc.vector.tensor_copy(dst[:, 1 + 3 * t:4 + 3 * t, 1:1 + W], ps)

        def k_conv_layer(srcs, wts, dst):
            n_chunks = len(wts)
            for t in range(4):
                ps = ps_k.tile([128, 10, S], F32, tag="pk", name="pk")
                first = True
                for ci in range(n_chunks):
                    for tap in range(9):
                        dy, dx = divmod(tap, 3)
                        nc.tensor.matmul(
                            ps, wts[ci][:, tap, :],
                            srcs[ci][:, 10 * t + dy:10 * t + dy + 10, dx:dx + S],
                            start=first,
                            stop=(ci == n_chunks - 1 and tap == 8))
                        first = False
                nc.vector.tensor_copy(dst[:, 1 + 10 * t:11 + 10 * t, 1:1 + S], ps)

        # ------------- mask layer 1 (streamed from DRAM) + k layer 1 -------------
        fw0 = [load_w("fw0a", "fw"), load_w("fw0b", "fw"), load_w("fw0c", "fw")]
        kw0 = [load_w("kw0a", "kw"), load_w("kw0b", "kw"), load_w("kw0c", "kw")]

        l1_stages = {}

        def l1_src(t, ci):
            if (t, ci) not in l1_stages:
                p = 2 if ci == 2 else 128
                st = stage.tile([p, 5, COLS], ADT, tag=f"st{ci}", name=f"st{ci}")
                nc.sync.dma_start(out=st, in_=img[128 * ci:128 * ci + p,
                                                  3 * t:3 * t + 5, :])
                l1_stages[(t, ci)] = st
            return l1_stages[(t, ci)]

        mask_conv_layer(l1_src, fw0, X[0])
        ab = emit_gn(X[0], HALF, W, 8.0, mask_gb[:, 0:2], cc_d[0] if 0 in cc_set else None, "m")
        k_conv_layer([kinA, kinB, kinC], kw0, KB[0])
        abk = emit_gn(KB[0], S, S, 4.0, k_gb[:, 0:2], None, "k")
        apply_gn(KB[0], abk, S, S)
        apply_gn(X[0], ab, SLAB, W, chunk=12)

        # ------------- layers 2..4 -------------
        cur = 0
        kcur = 0
        for l in (1, 2, 3):
            fwl = [load_w(f"fw{l}", "fw")]
            kwl = [load_w(f"kw{l}", "kw")]

            def src(t, ci, _cur=cur):
                return X[_cur][:, 3 * t:3 * t + 5, :]

            mask_conv_layer(src, fwl, X[1 - cur])
            ab = emit_gn(X[1 - cur], HALF, W, 8.0,
                         mask_gb[:, 2 * l:2 * l + 2], cc_d[l] if l in cc_set else None, "m")
            k_conv_layer([KB[kcur]], kwl, KB[1 - kcur])
            abk = emit_gn(KB[1 - kcur], S, S, 4.0, k_gb[:, 2 * l:2 * l + 2],
                          None, "k")
            apply_gn(KB[1 - kcur], abk, S, S)
            apply_gn(X[1 - cur], ab, SLAB, W, chunk=12)
            cur, kcur = 1 - cur, 1 - kcur

        khid = KB[kcur]   # l4 output (kcur flipped 3 times: KB[1])

        # ------------- fow + GN + relu -------------
        mf = X[1 - cur]
        for t in range(27):
            ps = ps_conv.tile([128, 3, W], F32, tag="pc", name="pc")
            nc.tensor.matmul(ps, fowT[:],
                             X[cur][:, 1 + 3 * t:4 + 3 * t, 1:1 + W],
                             start=True, stop=True)
            nc.vector.tensor_copy(mf[:, 1 + 3 * t:4 + 3 * t, 1:1 + W], ps)
        # ------------- gather + kow (fills fow's AllReduce bubble) -------------
        hidc = kpool.tile([128, S, S], F32, tag="KB0", name="KB0")  # reuse KB0's slot
        nc.vector.tensor_copy(hidc[:], khid[:, 1:1 + S, 1:1 + S].bitcast(F32))
        hidcf = hidc.rearrange("p a b -> p (a b)")
        hidT = kpool.tile([128, NCH, 128], F32, tag="sh_hidT", name="sh_hidT")
        for j in range(NCH):
            npos = min(128, NPOS - 128 * j)
            pt = ps_small.tile([128, 128], F32, tag="pss", name="pss")
            nc.tensor.transpose(pt[0:npos, :], hidcf[:, 128 * j:128 * j + npos],
                                ident)
            nc.vector.tensor_copy(hidT[0:npos, j, :], pt[0:npos, :])
        psel = ps_small.tile([128, KSEL], F32, tag="pss", name="pss")
        selPv = selP.rearrange("p (j k) -> p j k", j=NCH)
        for j in range(NCH):
            npos = min(128, NPOS - 128 * j)
            nc.tensor.matmul(psel, hidT[0:npos, j, :], selPv[0:npos, j, :],
                             start=(j == 0), stop=(j == NCH - 1))
        hsel = small.tile([128, KSEL], F32, tag="hsel", name="hsel")
        nc.vector.tensor_copy(hsel[:], psel)

        kowT = kpool.tile([128, C * L], F32, tag="sh_kow", name="sh_kow")
        nc.sync.dma_start(out=kowT, in_=kowT_d[:, :])
        kernT = small.tile([128, KL], F32, tag="kernT", name="kernT")
        kernTv = kernT.rearrange("p (k l) -> p k l", l=L)
        for l in range(L):
            pk = ps_small.tile([128, KSEL], F32, tag="pss", name="pss")
            nc.tensor.matmul(pk, kowT[:, 128 * l:128 * l + 128], hsel[:],
                             start=True, stop=True)
            nc.vector.tensor_scalar_add(kernTv[:, :, l], pk, kobT[:, l:l + 1])
        kernTr = small.tile([128, KL], ADT, tag="kernTr", name="kernTr")
        nc.vector.tensor_copy(kernTr[:], kernT[:])

        ab = emit_gn(mf, HALF, W, 8.0, mask_gb[:, 8:10], cc_d[4] if 4 in cc_set else None, "m")
        apply_gn(mf, ab, HALF + 1, W, chunk=12)

        # ------------- einsum + output -------------
        for (m0, mm) in ((0, 128), (128, 128), (256, 128), (384, KL - 384)):
            for t in range(27):
                rows = 3 if t < 26 else 2
                ps = ps_conv.tile([128, 3, W], F32, tag="pc", name="pc")
                nc.tensor.matmul(
                    ps[0:mm, 0:rows, :], kernTr[:, m0:m0 + mm],
                    mf[:, 1 + 3 * t:1 + 3 * t + rows, 1:1 + W],
                    start=True, stop=True)
                st = stage.tile([128, 3, W], F32, tag="st0", name="st0")
                if t % 2 == 0:
                    nc.vector.tensor_copy(st[0:mm, 0:rows, :], ps[0:mm, 0:rows, :])
                else:
                    nc.scalar.copy(out=st[0:mm, 0:rows, :], in_=ps[0:mm, 0:rows, :])
                nc.sync.dma_start(
                    out=out_d[m0:m0 + mm, 3 * W * t:3 * W * t + rows * W],
                    in_=st[0:mm, 0:rows, :])

    nc.compile()
    _PROGRAM_CACHE[key] = nc
    return nc


# --------------------------------------------------------------------------
# host glue
# --------------------------------------------------------------------------

def _prep_inputs(inputs, use_f32r=True):
    feats = np.asarray(inputs['feats'], np.float32)
    bboxes = np.asarray(inputs['matched_bboxes'])
    idx = topk_idx(bboxes, KSEL)

    rnd = _round_f32r if use_f32r else (lambda x: np.ascontiguousarray(x, np.float32))

    xx, yy = np.meshgrid(np.linspace(-1, 1, W, dtype=np.float64),
                         np.linspace(-1, 1, H, dtype=np.float64), indexing='xy')
    coord = np.stack([xx, yy]).astype(np.float32)
    Rh = resize_weight_mat(H, S).astype(np.float32)
    Rw = resize_weight_mat(W, S).astype(np.float32)

    def conv_w_prep(w, flip):
        w = np.asarray(w, np.float32)
        if flip:
            w = w[:, :, ::-1, :]
        return np.ascontiguousarray(
            w.transpose(1, 2, 3, 0).reshape(w.shape[1], 9, w.shape[0]))

    def gb_pack(pairs):
        out = np.zeros((128, 2 * len(pairs)), np.float32)
        for i, (g, b) in enumerate(pairs):
            out[:, 2 * i] = np.asarray(g, np.float32)
            out[:, 2 * i + 1] = np.asarray(b, np.float32)
        return out

    mask_gb = gb_pack([(inputs[f'fg{l}'], inputs[f'fb{l}']) for l in range(4)]
                      + [(inputs['fog'], inputs['fob'])])
    k_gb = gb_pack([(inputs[f'kg{l}'], inputs[f'kb{l}']) for l in range(4)])

    kow = np.asarray(inputs['kow'], np.float32).reshape(C * L, C)
    kowT = np.ascontiguousarray(kow.T)
    kob = np.asarray(inputs['kob'], np.float32)
    kobT = np.ascontiguousarray(kob.reshape(L, C).T)
    fow = np.asarray(inputs['fow'], np.float32).reshape(C, C)
    fowT = rnd(np.ascontiguousarray(fow.T))

    onehot_g = np.zeros((128, 32), np.float32)
    onehot_g[np.arange(128), np.arange(128) // 4] = 1.0
    onehot_b = np.ascontiguousarray(onehot_g.T)

    per_orient = {}
    for flip in (False, True):
        wd = {}
        for pre, key in (("f", "fw"), ("k", "kw")):
            w0 = conv_w_prep(inputs[f'{key}0'], flip)
            wd[f"{pre}w0a"] = rnd(w0[0:128])
            wd[f"{pre}w0b"] = rnd(w0[128:256])
            wd[f"{pre}w0c"] = rnd(w0[256:258])
            for l in (1, 2, 3):
                wd[f"{pre}w{l}"] = rnd(conv_w_prep(inputs[f'{key}{l}'], flip))
        per_orient[flip] = wd

    in_maps = []
    for c in range(8):
        b, s = c // 2, c % 2
        base = np.concatenate([feats[b], coord], 0)
        if s == 1:
            base = base[:, ::-1, :]
        img_pad = np.zeros((CINP, ROWS, COLS), np.float32)
        img_pad[:, 1:, 1:1 + W] = base[:, 0:ROWS - 1, :]

        t = np.tensordot(Rh, base, axes=(1, 1))
        kin = np.tensordot(t, Rw, axes=(2, 1)).transpose(1, 0, 2)
        kin_pad = np.zeros((CINP, GRID_P, GRID_P), np.float32)
        kin_pad[:, 1:-1, 1:-1] = kin

        selP = np.zeros((128, NCH * KSEL), np.float32)
        for k in range(KSEL):
            y_, x_ = divmod(int(idx[b, k]), S)
            if s == 1:
                y_ = S - 1 - y_
            p = y_ * S + x_
            selP[p % 128, (p // 128) * KSEL + k] = 1.0

        m = dict(per_orient[s == 1])
        m.update(
            img=rnd(img_pad), kin=rnd(kin_pad),
            fowT=fowT, kowT=kowT, mask_gb=mask_gb, k_gb=k_gb, kobT=kobT,
            onehot_g=onehot_g, onehot_b=onehot_b, selP=selP)
        in_maps.append(m)
    return in_maps


def assemble_output(results):
    out = np.zeros((B, KL, H, W), np.float32)
    for c in range(8):
        b, s = c // 2, c % 2
        pred = results[c]["out"].reshape(KL, HALF, W)
        if s == 0:
            out[b, :, 0:HALF, :] = pred
        else:
            out[b, :, HALF:, :] = pred[:, ::-1, :]
    return out.reshape(B, KSEL, L, H, W)


def kernel(**inputs) -> np.ndarray:
    use_f32r = True
    nc = build_program(use_f32r=use_f32r)
    in_maps = _prep_inputs(inputs, use_f32r=use_f32r)
    res = run_bass_kernel_spmd(nc, in_maps, core_ids=list(range(8)))
    return assemble_output(res.results)



# revision 27
# speedup vs baseline: 1.3827x; 1.3827x over previous
"""Trainium2 Bass kernel for nn_CharacterMaskAttentionHead.

Sharding: 8 cores = 4 images x 2 H-halves (data-parallel over batch, spatial
split within each image). Bottom halves are H-flipped on the host so every core
runs the identical SPMD program on a "top slab" (true image edge at the top,
4-row interior halo at the bottom). GroupNorm statistics are exact: each core
reduces its half, per-pair partial sums are combined with a tiny (32x2 float)
in-kernel AllReduce that overlaps with kernel-branch compute.

Optimizations over the first working version:
  - layer-1 coord channels folded into a host-precomputed bias map that the
    PSUM drain adds (saves 252 matmuls/core of a 2-partition chunk);
  - GN stats read directly from PSUM, one bn_stats per conv tile, issued
    incrementally during the conv so the stats AllReduce can trigger
    immediately after the last tile;
  - emit_gn split into start (stats + CC trigger) / finish (CC consume) with
    the kernel-tower convs (or the gather+kow chain for the final GN) queued
    in between, so the PE never stalls on the AllReduce;
  - einsum output staged as fp16, 6 tiles per DMA, alternating between the
    sync and gpsimd DMA queues (the f32 single-queue version was tail-bound);
  - DMA issue order: layer-1 weights + first row tiles first; constants, kin
    and the stats CC round-trips ride the gpsimd queue, the coord-bias stream
    rides the scalar queue.
"""
import sys
import contextlib

sys.path.insert(0, '/opt/trn_rl_repo')

import numpy as np

import concourse.bacc as bacc
import concourse.bass as bass
import concourse.tile as tile
from concourse import mybir
from concourse.bass_utils import run_bass_kernel_spmd
from concourse.masks import make_identity

F32 = mybir.dt.float32
F32R = mybir.dt.float32r
F16 = mybir.dt.float16

B, CIN, H, W = 4, 256, 160, 160
C, L, S, KSEL = 128, 25, 40, 16
CINP = CIN + 2
HALF = 80                 # output rows per core
SLAB = 84                 # conv rows per layer (80 + 4 halo)
ROWS, COLS = 86, 162      # padded slab buffer
GRID_P = S + 2            # padded 40x40 grid (42)
NT = 28                   # mask conv tiles (3 rows x 160)
KL = KSEL * L             # 400
NPOS = S * S              # 1600
NCH = 13                  # ceil(1600/128)
EPS = 1e-5

_PROGRAM_CACHE = {}


def _round_f32r(x):
    """Round fp32 to the f32r grid (12 mantissa bits) so DMA-fed float32r
    tiles hold values the PE will not re-round differently in sim vs HW."""
    b = np.ascontiguousarray(x, np.float32).view(np.uint32)
    b = ((b + 0x800) & np.uint32(0xFFFFF000)).astype(np.uint32)
    return b.view(np.float32)


def resize_weight_mat(in_size, out_size):
    """jax.image.resize bilinear+antialias weight matrix; out = Wmat @ in."""
    inv_scale = in_size / out_size
    kernel_scale = max(inv_scale, 1.0)
    sample_f = (np.arange(out_size) + 0.5) * inv_scale - 0.5
    x = np.abs(sample_f[:, None] - np.arange(in_size)[None, :]) / kernel_scale
    w = np.maximum(0.0, 1.0 - x)
    tot = w.sum(axis=1, keepdims=True)
    w = np.where(np.abs(tot) > 1000 * np.finfo(np.float32).eps, w / tot, 0.0)
    valid = (sample_f >= -0.5) & (sample_f <= in_size - 0.5)
    return np.where(valid[:, None], w, 0.0)


def topk_idx(vals, k):
    """jax.lax.top_k indices: descending value, ties -> lower index first."""
    return np.argsort(-vals, axis=-1, kind='stable')[..., :k]


# --------------------------------------------------------------------------
# device program
# --------------------------------------------------------------------------

def build_program(use_f32r=True, use_cc=True):
    cc_set = (set(range(5)) if use_cc is True
              else (set() if use_cc is False else set(use_cc)))
    key = (use_f32r, tuple(sorted(cc_set)))
    if key in _PROGRAM_CACHE:
        return _PROGRAM_CACHE[key]
    ADT = F32R if use_f32r else F32

    nc = bacc.Bacc("TRN2", target_bir_lowering=False, num_devices=8)

    img = nc.dram_tensor("img", (CIN, ROWS, COLS), ADT, kind="ExternalInput")
    kin = nc.dram_tensor("kin", (CIN, GRID_P, GRID_P), ADT, kind="ExternalInput")
    cbias_d = nc.dram_tensor("cbias", (128, SLAB, W), ADT, kind="ExternalInput")
    kbias_d = nc.dram_tensor("kbias", (128, S, S), ADT, kind="ExternalInput")
    wdr = {}
    for pre in ("f", "k"):
        wdr[f"{pre}w0a"] = nc.dram_tensor(f"{pre}w0a", (128, 9, 128), ADT, kind="ExternalInput")
        wdr[f"{pre}w0b"] = nc.dram_tensor(f"{pre}w0b", (128, 9, 128), ADT, kind="ExternalInput")
        for l in (1, 2, 3):
            wdr[f"{pre}w{l}"] = nc.dram_tensor(f"{pre}w{l}", (128, 9, 128), ADT, kind="ExternalInput")
    fowT_d = nc.dram_tensor("fowT", (128, 128), ADT, kind="ExternalInput")
    kowT_d = nc.dram_tensor("kowT", (128, C * L), F32, kind="ExternalInput")
    mask_gb_d = nc.dram_tensor("mask_gb", (128, 10), F32, kind="ExternalInput")
    k_gb_d = nc.dram_tensor("k_gb", (128, 8), F32, kind="ExternalInput")
    kobT_d = nc.dram_tensor("kobT", (128, L), F32, kind="ExternalInput")
    onehot_g_d = nc.dram_tensor("onehot_g", (128, 32), F32, kind="ExternalInput")
    onehot_b_d = nc.dram_tensor("onehot_b", (32, 128), F32, kind="ExternalInput")
    selP_d = nc.dram_tensor("selP", (128, NCH * KSEL), F32, kind="ExternalInput")
    out_d = nc.dram_tensor("out", (KL, HALF * W), F16, kind="ExternalOutput")
    cc_d = [(nc.dram_tensor(f"ccin{l}", (32, 2), F32),
             nc.dram_tensor(f"ccout{l}", (32, 2), F32)) for l in range(5)]

    with tile.TileContext(nc) as tc, contextlib.ExitStack() as ctx:
        consts = ctx.enter_context(tc.tile_pool(name="consts", bufs=1))
        acts = ctx.enter_context(tc.tile_pool(name="acts", bufs=1))
        wpool = ctx.enter_context(tc.tile_pool(name="wpool", bufs=3))
        small = ctx.enter_context(tc.tile_pool(name="small", bufs=1))
        stage = ctx.enter_context(tc.tile_pool(name="stage", bufs=2))
        bpool = ctx.enter_context(tc.tile_pool(name="bpool", bufs=2))
        ost = ctx.enter_context(tc.tile_pool(name="ost", bufs=2))
        kpool = ctx.enter_context(tc.tile_pool(name="kpool", bufs=1))
        ps_conv = ctx.enter_context(tc.tile_pool(name="ps_conv", bufs=4, space="PSUM"))
        ps_k = ctx.enter_context(tc.tile_pool(name="ps_k", bufs=2, space="PSUM"))
        ps_small = ctx.enter_context(tc.tile_pool(name="ps_small", bufs=2, space="PSUM"))

        # ------------- layer-1 weights first on the sync queue -------------
        def load_w(nm, tag, eng=None):
            dr = wdr[nm]
            t = wpool.tile([dr.shape[0], 9, 128], ADT, tag=tag, name=nm)
            (eng or nc.sync).dma_start(out=t, in_=dr[:, :, :])
            return t

        fw0 = [load_w("fw0a", "fw"), load_w("fw0b", "fw")]

        # ------------- constants (gpsimd queue; tiny) -------------
        def load_const(dram, shape, dt=F32):
            t = consts.tile(shape, dt, tag=dram.name, name=dram.name)
            nc.gpsimd.dma_start(out=t, in_=dram[tuple(slice(None) for _ in shape)])
            return t

        onehot_g = load_const(onehot_g_d, [128, 32])
        onehot_b = load_const(onehot_b_d, [32, 128])
        mask_gb = load_const(mask_gb_d, [128, 10])
        k_gb = load_const(k_gb_d, [128, 8])
        kobT = load_const(kobT_d, [128, L])
        selP = load_const(selP_d, [128, NCH * KSEL])
        fowT = load_const(fowT_d, [128, 128], ADT)
        eps32 = consts.tile([32, 1], F32, tag="eps32", name="eps32")
        nc.vector.memset(eps32, EPS)
        ident = consts.tile([128, 128], F32, tag="ident", name="ident")
        make_identity(nc, ident)
        # f32r identity: adds host bias maps into PSUM at 1 cycle/row
        ident_r = consts.tile([128, 128], ADT, tag="ident_r", name="ident_r")
        nc.vector.tensor_copy(ident_r[:], ident[:])

        # activation slabs (ping-pong); pads zeroed via tensor_copy from an
        # F32 zero tile (memset cannot produce f32r, tensor_copy can)
        zsrc = consts.tile([128, COLS], F32, tag="zsrc", name="zsrc")
        nc.vector.memset(zsrc, 0.0)
        zcol = zsrc[:, 0:ROWS].rearrange("p (a b) -> p a b", b=1)

        X = [acts.tile([128, ROWS, COLS], ADT, tag="X0", name="X0"),
             acts.tile([128, ROWS, COLS], ADT, tag="X1", name="X1")]
        for x in X:
            nc.vector.tensor_copy(x[:, 0, :], zsrc[:, :])
            nc.vector.tensor_copy(x[:, ROWS - 1, :], zsrc[:, :])
            nc.vector.tensor_copy(x[:, :, 0:1], zcol)
            nc.vector.tensor_copy(x[:, :, COLS - 1:COLS], zcol)
        KB = [kpool.tile([128, GRID_P, GRID_P], ADT, tag="KB0", name="KB0"),
              kpool.tile([128, GRID_P, GRID_P], ADT, tag="KB1", name="KB1")]
        zcolk = zsrc[:, 0:GRID_P].rearrange("p (a b) -> p a b", b=1)
        for x in KB:
            nc.vector.tensor_copy(x[:, 0, :], zsrc[:, 0:GRID_P])
            nc.vector.tensor_copy(x[:, GRID_P - 1, :], zsrc[:, 0:GRID_P])
            nc.vector.tensor_copy(x[:, :, 0:1], zcolk)
            nc.vector.tensor_copy(x[:, :, GRID_P - 1:GRID_P], zcolk)

        # kin chunks (tags shared with later k-branch tiles to save SBUF)
        kinA = kpool.tile([128, GRID_P, GRID_P], ADT, tag="sh_kow", name="sh_kow")
        kinB = kpool.tile([128, GRID_P, GRID_P], ADT, tag="sh_hidT", name="sh_hidT")
        nc.gpsimd.dma_start(out=kinA, in_=kin[0:128, :, :])
        nc.gpsimd.dma_start(out=kinB, in_=kin[128:256, :, :])

        # ------------- GroupNorm (split emitters) -------------
        # stats are accumulated per-conv-tile straight from PSUM into a
        # [128, ntiles, 6] tile; gn_start aggregates + triggers the pair CC,
        # gn_finish consumes the CC result (PE work goes in between).
        def stats_tile(fam, ntiles):
            return small.tile([128, ntiles, 6], F32, tag=f"bnst{fam}",
                              name=f"bnst{fam}")

        def gn_start(stats, divisor, cc, fam):
            mv = small.tile([128, 2], F32, tag=f"mv{fam}", name=f"mv{fam}")
            nc.vector.bn_aggr(out=mv[:], in_=stats.rearrange("p a b -> p (a b)"))
            sq = small.tile([128, 2], F32, tag=f"sq{fam}", name=f"sq{fam}")
            nc.vector.tensor_mul(sq[:, 1:2], mv[:, 0:1], mv[:, 0:1])
            nc.vector.tensor_add(sq[:, 1:2], sq[:, 1:2], mv[:, 1:2])
            nc.vector.tensor_copy(sq[:, 0:1], mv[:, 0:1])

            pg = ps_small.tile([32, 2], F32, tag="pss", name="pss")
            nc.tensor.matmul(pg, onehot_g[:], sq[:], start=True, stop=True)
            g32 = small.tile([32, 2], F32, tag=f"g32{fam}", name=f"g32{fam}")
            nc.vector.tensor_copy(g32[:], pg)
            if cc is not None:
                ccin_d, ccout_d = cc
                nc.gpsimd.dma_start(out=ccin_d[:, :], in_=g32[:])
                nc.gpsimd.collective_compute(
                    "AllReduce", mybir.AluOpType.add,
                    replica_groups=[[0, 1], [2, 3], [4, 5], [6, 7]],
                    ins=[ccin_d[:, :]], outs=[ccout_d[:, :]])
            return g32

        def gn_finish(g32, divisor, gb_ap, cc, fam):
            if cc is not None:
                _, ccout_d = cc
                g32 = small.tile([32, 2], F32, tag=f"g32r{fam}", name=f"g32r{fam}")
                nc.gpsimd.dma_start(out=g32[:], in_=ccout_d[:, :])
            nc.vector.tensor_scalar_mul(g32[:], g32[:], 1.0 / divisor)
            msq = small.tile([32, 1], F32, tag=f"msq{fam}", name=f"msq{fam}")
            nc.vector.tensor_mul(msq[:], g32[:, 0:1], g32[:, 0:1])
            nc.vector.tensor_sub(g32[:, 1:2], g32[:, 1:2], msq[:])
            nc.scalar.activation(out=g32[:, 1:2], in_=g32[:, 1:2],
                                 func=mybir.ActivationFunctionType.Sqrt,
                                 bias=eps32[:], scale=1.0)
            nc.vector.reciprocal(g32[:, 1:2], g32[:, 1:2])

            pb = ps_small.tile([128, 2], F32, tag="pss", name="pss")
            nc.tensor.matmul(pb, onehot_b[:], g32[:], start=True, stop=True)
            mr = small.tile([128, 2], F32, tag=f"mr{fam}", name=f"mr{fam}")
            nc.vector.tensor_copy(mr[:], pb)

            ab = small.tile([128, 2], F32, tag=f"ab{fam}", name=f"ab{fam}")
            nc.vector.tensor_mul(ab[:, 0:1], gb_ap[:, 0:1], mr[:, 1:2])
            nc.vector.tensor_mul(ab[:, 1:2], mr[:, 0:1], ab[:, 0:1])
            nc.vector.tensor_sub(ab[:, 1:2], gb_ap[:, 1:2], ab[:, 1:2])
            return ab

        def apply_gn(dst, ab, nrows, ncols, chunks=(6, 18, 18, 18, 18, 18)):
            r = 0
            ci = 0
            while r < nrows:
                take = min(chunks[ci] if ci < len(chunks) else 18, nrows - r)
                nc.scalar.activation(
                    out=dst[:, 1 + r:1 + r + take, 1:1 + ncols],
                    in_=dst[:, 1 + r:1 + r + take, 1:1 + ncols].bitcast(F32),
                    func=mybir.ActivationFunctionType.Relu,
                    scale=ab[:, 0:1], bias=ab[:, 1:2])
                r += take
                ci += 1

        # ------------- conv emitters -------------
        def mask_conv_layer(src_fn, wts, dst, stats, bias_fn=None):
            """28 tiles of 3 rows; one bn_stats per tile straight from PSUM
            (valid rows only) so the CC can trigger as soon as the last tile
            finishes.  A host bias map is accumulated into PSUM via an f32r
            identity matmul (so stats include it)."""
            n_chunks = len(wts)
            for t in range(NT):
                if bias_fn is not None:
                    btile = bias_fn(t)
                ps = ps_conv.tile([128, 3, W], F32, tag="pc", name="pc")
                first = True
                for ci in range(n_chunks):
                    rhs_tile = src_fn(t, ci)
                    for tap in range(9):
                        dy, dx = divmod(tap, 3)
                        last = (ci == n_chunks - 1 and tap == 8
                                and bias_fn is None)
                        nc.tensor.matmul(
                            ps, wts[ci][:, tap, :],
                            rhs_tile[:, dy:dy + 3, dx:dx + W],
                            start=first, stop=last)
                        first = False
                if bias_fn is not None:
                    nc.tensor.matmul(ps, ident_r[:], btile, start=False,
                                     stop=True)
                # stats: tiles 0..25 all 3 rows, tile 26 first 2 rows
                if t < 26:
                    nc.vector.bn_stats(out=stats[:, t, :],
                                       in_=ps.rearrange("p a b -> p (a b)"))
                elif t == 26:
                    nc.vector.bn_stats(
                        out=stats[:, t, :],
                        in_=ps[:, 0:2, :].rearrange("p a b -> p (a b)"))
                nc.vector.tensor_copy(dst[:, 1 + 3 * t:4 + 3 * t, 1:1 + W], ps)

        def k_conv_layer(srcs, wts, dst, stats, bias_fn=None):
            n_chunks = len(wts)
            for t in range(4):
                if bias_fn is not None:
                    btile = bias_fn(t)
                ps = ps_k.tile([128, 10, S], F32, tag="pk", name="pk")
                first = True
                for ci in range(n_chunks):
                    for tap in range(9):
                        dy, dx = divmod(tap, 3)
                        last = (ci == n_chunks - 1 and tap == 8
                                and bias_fn is None)
                        nc.tensor.matmul(
                            ps, wts[ci][:, tap, :],
                            srcs[ci][:, 10 * t + dy:10 * t + dy + 10, dx:dx + S],
                            start=first, stop=last)
                        first = False
                if bias_fn is not None:
                    nc.tensor.matmul(ps, ident_r[:], btile, start=False,
                                     stop=True)
                nc.vector.bn_stats(out=stats[:, t, :],
                                   in_=ps.rearrange("p a b -> p (a b)"))
                nc.vector.tensor_copy(dst[:, 1 + 10 * t:11 + 10 * t, 1:1 + S],
                                      ps)

        # ------------- mask layer 1 (streamed) + k layer 1 -------------
        l1_stages = {}

        def l1_src(t, ci):
            if (t, ci) not in l1_stages:
                st = stage.tile([128, 5, COLS], ADT, tag=f"st{ci}", name=f"st{ci}")
                nc.sync.dma_start(out=st, in_=img[128 * ci:128 * ci + 128,
                                                  3 * t:3 * t + 5, :])
                l1_stages[(t, ci)] = st
            return l1_stages[(t, ci)]

        def l1_bias(t):
            bt = bpool.tile([128, 3, W], ADT, tag="cb", name="cb")
            nc.scalar.dma_start(out=bt, in_=cbias_d[:, 3 * t:3 * t + 3, :])
            return bt

        def k1_bias(t):
            bt = bpool.tile([128, 10, S], ADT, tag="cb", name="cb")
            nc.scalar.dma_start(out=bt, in_=kbias_d[:, 10 * t:10 * t + 10, :])
            return bt

        # k layer-1 weights early on the gpsimd queue (sync is busy streaming)
        kw0 = [load_w("kw0a", "kw", eng=nc.gpsimd),
               load_w("kw0b", "kw", eng=nc.gpsimd)]

        st_m = stats_tile("m", 27)
        mask_conv_layer(l1_src, fw0, X[0], st_m, bias_fn=l1_bias)
        g32m = gn_start(st_m, 8.0, cc_d[0] if 0 in cc_set else None, "m")

        st_k = stats_tile("k", 4)
        k_conv_layer([kinA, kinB], kw0, KB[0], st_k, bias_fn=k1_bias)
        g32k = gn_start(st_k, 4.0, None, "k")
        abk = gn_finish(g32k, 4.0, k_gb[:, 0:2], None, "k")
        apply_gn(KB[0], abk, S, S, chunks=(40,))
        ab = gn_finish(g32m, 8.0, mask_gb[:, 0:2],
                       cc_d[0] if 0 in cc_set else None, "m")
        apply_gn(X[0], ab, SLAB, W)

        # ------------- layers 2..4 -------------
        cur = 0
        kcur = 0
        for l in (1, 2, 3):
            fwl = [load_w(f"fw{l}", "fw")]
            kwl = [load_w(f"kw{l}", "kw")]

            def src(t, ci, _cur=cur):
                return X[_cur][:, 3 * t:3 * t + 5, :]

            st_m = stats_tile("m", 27)
            mask_conv_layer(src, fwl, X[1 - cur], st_m)
            g32m = gn_start(st_m, 8.0, cc_d[l] if l in cc_set else None, "m")
            st_k = stats_tile("k", 4)
            k_conv_layer([KB[kcur]], kwl, KB[1 - kcur], st_k)
            g32k = gn_start(st_k, 4.0, None, "k")
            abk = gn_finish(g32k, 4.0, k_gb[:, 2 * l:2 * l + 2], None, "k")
            apply_gn(KB[1 - kcur], abk, S, S, chunks=(40,))
            ab = gn_finish(g32m, 8.0, mask_gb[:, 2 * l:2 * l + 2],
                           cc_d[l] if l in cc_set else None, "m")
            apply_gn(X[1 - cur], ab, SLAB, W)
            if l == 1:
                # kowT into kinA's slot (free since k layer-1); emitted here
                # so its gpsimd-queue slot sits after ccout1, long before use
                kowT = kpool.tile([128, C * L], F32, tag="sh_kow", name="sh_kow")
                nc.gpsimd.dma_start(out=kowT, in_=kowT_d[:, :])
            cur, kcur = 1 - cur, 1 - kcur

        khid = KB[kcur]   # l4 output (kcur flipped 3 times: KB[1])

        # ------------- fow conv (stats from PSUM) -------------
        mf = X[1 - cur]
        st_f = stats_tile("m", 27)
        for t in range(27):
            ps = ps_conv.tile([128, 3, W], F32, tag="pc", name="pc")
            nc.tensor.matmul(ps, fowT[:],
                             X[cur][:, 1 + 3 * t:4 + 3 * t, 1:1 + W],
                             start=True, stop=True)
            if t < 26:
                nc.vector.bn_stats(out=st_f[:, t, :],
                                   in_=ps.rearrange("p a b -> p (a b)"))
            else:
                nc.vector.bn_stats(
                    out=st_f[:, t, :],
                    in_=ps[:, 0:2, :].rearrange("p a b -> p (a b)"))
            nc.vector.tensor_copy(mf[:, 1 + 3 * t:4 + 3 * t, 1:1 + W], ps)
        g32f = gn_start(st_f, 8.0, cc_d[4] if 4 in cc_set else None, "m")

        # ------------- gather + kow (fills fow's AllReduce window) -------------
        hidc = kpool.tile([128, S, S], F32, tag="KB0", name="KB0")  # reuse KB0
        nc.vector.tensor_copy(hidc[:], khid[:, 1:1 + S, 1:1 + S].bitcast(F32))
        hidcf = hidc.rearrange("p a b -> p (a b)")
        hidT = kpool.tile([128, NCH, 128], F32, tag="sh_hidT", name="sh_hidT")
        for j in range(NCH):
            npos = min(128, NPOS - 128 * j)
            pt = ps_small.tile([128, 128], F32, tag="pss", name="pss")
            nc.tensor.transpose(pt[0:npos, :], hidcf[:, 128 * j:128 * j + npos],
                                ident)
            nc.vector.tensor_copy(hidT[0:npos, j, :], pt[0:npos, :])
        psel = ps_small.tile([128, KSEL], F32, tag="pss", name="pss")
        selPv = selP.rearrange("p (j k) -> p j k", j=NCH)
        for j in range(NCH):
            npos = min(128, NPOS - 128 * j)
            nc.tensor.matmul(psel, hidT[0:npos, j, :], selPv[0:npos, j, :],
                             start=(j == 0), stop=(j == NCH - 1))
        hsel = small.tile([128, KSEL], F32, tag="hsel", name="hsel")
        nc.vector.tensor_copy(hsel[:], psel)

        kernT = small.tile([128, KL], F32, tag="kernT", name="kernT")
        kernTv = kernT.rearrange("p (k l) -> p k l", l=L)
        for l in range(L):
            pk = ps_small.tile([128, KSEL], F32, tag="pss", name="pss")
            nc.tensor.matmul(pk, kowT[:, 128 * l:128 * l + 128], hsel[:],
                             start=True, stop=True)
            nc.vector.tensor_scalar_add(kernTv[:, :, l], pk, kobT[:, l:l + 1])
        kernTr = small.tile([128, KL], ADT, tag="kernTr", name="kernTr")
        nc.vector.tensor_copy(kernTr[:], kernT[:])

        ab = gn_finish(g32f, 8.0, mask_gb[:, 8:10],
                       cc_d[4] if 4 in cc_set else None, "m")
        apply_gn(mf, ab, HALF + 1, W)

        # ------------- einsum + fp16 output (6 conv tiles per DMA) -------------
        groups = [(0, 6), (6, 6), (12, 6), (18, 6), (24, 3)]
        for gi, (m0, mm) in enumerate(((0, 128), (128, 128), (256, 128),
                                       (384, KL - 384))):
            for t0, gn_ in groups:
                grows = sum(3 if t < 26 else 2 for t in range(t0, t0 + gn_))
                st = ost.tile([128, 18, W], F16, tag="ot", name="ot")
                for ti, t in enumerate(range(t0, t0 + gn_)):
                    rows = 3 if t < 26 else 2
                    ps = ps_conv.tile([128, 3, W], F32, tag="pc", name="pc")
                    nc.tensor.matmul(
                        ps[0:mm, 0:rows, :], kernTr[:, m0:m0 + mm],
                        mf[:, 1 + 3 * t:1 + 3 * t + rows, 1:1 + W],
                        start=True, stop=True)
                    if t % 2 == 0:
                        nc.vector.tensor_copy(
                            st[0:mm, 3 * ti:3 * ti + rows, :],
                            ps[0:mm, 0:rows, :])
                    else:
                        nc.scalar.copy(out=st[0:mm, 3 * ti:3 * ti + rows, :],
                                       in_=ps[0:mm, 0:rows, :])
                eng = nc.sync if (t0 // 6) % 2 == 0 else nc.gpsimd
                eng.dma_start(
                    out=out_d[m0:m0 + mm,
                              3 * W * t0:3 * W * t0 + grows * W],
                    in_=st[0:mm, 0:grows, :])

    nc.compile()
    _PROGRAM_CACHE[key] = nc
    return nc


# --------------------------------------------------------------------------
# host glue
# --------------------------------------------------------------------------

def _prep_inputs(inputs, use_f32r=True):
    feats = np.asarray(inputs['feats'], np.float32)
    bboxes = np.asarray(inputs['matched_bboxes'])
    idx = topk_idx(bboxes, KSEL)

    rnd = _round_f32r if use_f32r else (lambda x: np.ascontiguousarray(x, np.float32))

    xx, yy = np.meshgrid(np.linspace(-1, 1, W, dtype=np.float64),
                         np.linspace(-1, 1, H, dtype=np.float64), indexing='xy')
    coord = np.stack([xx, yy]).astype(np.float32)
    Rh = resize_weight_mat(H, S).astype(np.float32)
    Rw = resize_weight_mat(W, S).astype(np.float32)

    def conv_w_prep(w, flip):
        w = np.asarray(w, np.float32)
        if flip:
            w = w[:, :, ::-1, :]
        return np.ascontiguousarray(
            w.transpose(1, 2, 3, 0).reshape(w.shape[1], 9, w.shape[0]))

    def gb_pack(pairs):
        out = np.zeros((128, 2 * len(pairs)), np.float32)
        for i, (g, b) in enumerate(pairs):
            out[:, 2 * i] = np.asarray(g, np.float32)
            out[:, 2 * i + 1] = np.asarray(b, np.float32)
        return out

    mask_gb = gb_pack([(inputs[f'fg{l}'], inputs[f'fb{l}']) for l in range(4)]
                      + [(inputs['fog'], inputs['fob'])])
    k_gb = gb_pack([(inputs[f'kg{l}'], inputs[f'kb{l}']) for l in range(4)])

    kow = np.asarray(inputs['kow'], np.float32).reshape(C * L, C)
    kowT = np.ascontiguousarray(kow.T)
    kob = np.asarray(inputs['kob'], np.float32)
    kobT = np.ascontiguousarray(kob.reshape(L, C).T)
    fow = np.asarray(inputs['fow'], np.float32).reshape(C, C)
    fowT = rnd(np.ascontiguousarray(fow.T))

    onehot_g = np.zeros((128, 32), np.float32)
    onehot_g[np.arange(128), np.arange(128) // 4] = 1.0
    onehot_b = np.ascontiguousarray(onehot_g.T)

    per_orient = {}
    kw9c = {}
    for flip in (False, True):
        wd = {}
        for pre, key in (("f", "fw"), ("k", "kw")):
            w0 = conv_w_prep(inputs[f'{key}0'], flip)
            wd[f"{pre}w0a"] = rnd(w0[0:128])
            wd[f"{pre}w0b"] = rnd(w0[128:256])
            for l in (1, 2, 3):
                wd[f"{pre}w{l}"] = rnd(conv_w_prep(inputs[f'{key}{l}'], flip))
        # coord-channel contribution to the layer-1 conv output, as a
        # host-precomputed bias map [128, SLAB, W] (added during PSUM drain)
        coord_or = coord[:, ::-1, :] if flip else coord
        cslab = np.zeros((2, ROWS, COLS), np.float32)
        cslab[:, 1:, 1:1 + W] = coord_or[:, 0:ROWS - 1, :]
        w9c = rnd(conv_w_prep(inputs['fw0'], flip)[256:258])
        cb = np.zeros((128, SLAB, W), np.float32)
        for tap in range(9):
            dy, dx = divmod(tap, 3)
            cb += np.einsum('co,crw->orw', w9c[:, tap, :],
                            cslab[:, dy:dy + SLAB, dx:dx + W])
        wd["cbias"] = rnd(cb)
        kw9c[flip] = rnd(conv_w_prep(inputs['kw0'], flip)[256:258])
        per_orient[flip] = wd

    in_maps = []
    for c in range(8):
        b, s = c // 2, c % 2
        base = np.concatenate([feats[b], coord], 0)
        if s == 1:
            base = base[:, ::-1, :]
        img_pad = np.zeros((CINP, ROWS, COLS), np.float32)
        img_pad[:, 1:, 1:1 + W] = base[:, 0:ROWS - 1, :]

        t = np.tensordot(Rh, base, axes=(1, 1))
        kin = np.tensordot(t, Rw, axes=(2, 1)).transpose(1, 0, 2)
        kin_pad = np.zeros((CINP, GRID_P, GRID_P), np.float32)
        kin_pad[:, 1:-1, 1:-1] = kin
        # k-tower layer-1 coord contribution (per-core: depends on the
        # resized grid of this image/orientation)
        w9kc = kw9c[s == 1]
        kin_r = rnd(kin_pad)
        kb_ = np.zeros((128, S, S), np.float32)
        for kk in range(9):
            dy, dx = divmod(kk, 3)
            kb_ += np.einsum('co,crw->orw', w9kc[:, kk, :],
                             kin_r[256:258, dy:dy + S, dx:dx + S])

        selP = np.zeros((128, NCH * KSEL), np.float32)
        for k in range(KSEL):
            y_, x_ = divmod(int(idx[b, k]), S)
            if s == 1:
                y_ = S - 1 - y_
            p = y_ * S + x_
            selP[p % 128, (p // 128) * KSEL + k] = 1.0

        m = dict(per_orient[s == 1])
        m.update(
            img=rnd(img_pad)[0:CIN], kin=np.ascontiguousarray(kin_r[0:CIN]),
            kbias=rnd(kb_),
            fowT=fowT, kowT=kowT, mask_gb=mask_gb, k_gb=k_gb, kobT=kobT,
            onehot_g=onehot_g, onehot_b=onehot_b, selP=selP)
        in_maps.append(m)
    return in_maps


def assemble_output(results):
    out = np.zeros((B, KL, H, W), np.float32)
    for c in range(8):
        b, s = c // 2, c % 2
        pred = np.asarray(results[c]["out"], np.float32).reshape(KL, HALF, W)
        if s == 0:
            out[b, :, 0:HALF, :] = pred
        else:
            out[b, :, HALF:, :] = pred[:, ::-1, :]
    return out.reshape(B, KSEL, L, H, W)


def kernel(**inputs) -> np.ndarray:
    use_f32r = True
    nc = build_program(use_f32r=use_f32r)
    in_maps = _prep_inputs(inputs, use_f32r=use_f32r)
    res = run_bass_kernel_spmd(nc, in_maps, core_ids=list(range(8)))
    return assemble_output(res.results)


# revision 39
# speedup vs baseline: 1.4573x; 1.0539x over previous
"""Trainium2 Bass kernel for nn_CharacterMaskAttentionHead.

Sharding: 8 cores = 4 images x 2 H-halves (data-parallel over batch, spatial
split within each image). Bottom halves are H-flipped on the host so every core
runs the identical SPMD program on a "top slab" (true image edge at the top,
4-row interior halo at the bottom). GroupNorm statistics are exact: each core
reduces its half, per-pair partial sums are combined with a tiny (32x2 float)
in-kernel AllReduce that overlaps with kernel-branch compute.

Optimizations over the first working version:
  - layer-1 coord channels folded into a host-precomputed bias map that the
    PSUM drain adds (saves 252 matmuls/core of a 2-partition chunk);
  - GN stats read directly from PSUM, one bn_stats per conv tile, issued
    incrementally during the conv so the stats AllReduce can trigger
    immediately after the last tile;
  - emit_gn split into start (stats + CC trigger) / finish (CC consume) with
    the kernel-tower convs (or the gather+kow chain for the final GN) queued
    in between, so the PE never stalls on the AllReduce;
  - einsum output staged as fp16, 6 tiles per DMA, alternating between the
    sync and gpsimd DMA queues (the f32 single-queue version was tail-bound);
  - DMA issue order: layer-1 weights + first row tiles first; constants, kin
    and the stats CC round-trips ride the gpsimd queue, the coord-bias stream
    rides the scalar queue.
"""
import sys
import contextlib

sys.path.insert(0, '/opt/trn_rl_repo')

import numpy as np

import concourse.bacc as bacc
import concourse.bass as bass
import concourse.tile as tile
from concourse import mybir
from concourse.bass_utils import run_bass_kernel_spmd
from concourse.masks import make_identity

F32 = mybir.dt.float32
F32R = mybir.dt.float32r
F16 = mybir.dt.float16

B, CIN, H, W = 4, 256, 160, 160
C, L, S, KSEL = 128, 25, 40, 16
CINP = CIN + 2
HALF = 80                 # output rows per core
SLAB = 84                 # conv rows per layer (80 + 4 halo)
ROWS, COLS = 86, 162      # padded slab buffer
GRID_P = S + 2            # padded 40x40 grid (42)
NT = 28                   # mask conv tiles (3 rows x 160)
KL = KSEL * L             # 400
NPOS = S * S              # 1600
NCH = 13                  # ceil(1600/128)
EPS = 1e-5

_PROGRAM_CACHE = {}


def _round_f32r(x):
    """Round fp32 to the f32r grid (12 mantissa bits) so DMA-fed float32r
    tiles hold values the PE will not re-round differently in sim vs HW."""
    b = np.ascontiguousarray(x, np.float32).view(np.uint32)
    b = ((b + 0x800) & np.uint32(0xFFFFF000)).astype(np.uint32)
    return b.view(np.float32)


def resize_weight_mat(in_size, out_size):
    """jax.image.resize bilinear+antialias weight matrix; out = Wmat @ in."""
    inv_scale = in_size / out_size
    kernel_scale = max(inv_scale, 1.0)
    sample_f = (np.arange(out_size) + 0.5) * inv_scale - 0.5
    x = np.abs(sample_f[:, None] - np.arange(in_size)[None, :]) / kernel_scale
    w = np.maximum(0.0, 1.0 - x)
    tot = w.sum(axis=1, keepdims=True)
    w = np.where(np.abs(tot) > 1000 * np.finfo(np.float32).eps, w / tot, 0.0)
    valid = (sample_f >= -0.5) & (sample_f <= in_size - 0.5)
    return np.where(valid[:, None], w, 0.0)


def topk_idx(vals, k):
    """jax.lax.top_k indices: descending value, ties -> lower index first."""
    return np.argsort(-vals, axis=-1, kind='stable')[..., :k]


# --------------------------------------------------------------------------
# device program
# --------------------------------------------------------------------------

def build_program(use_f32r=True, use_cc=True):
    cc_set = (set(range(5)) if use_cc is True
              else (set() if use_cc is False else set(use_cc)))
    key = (use_f32r, tuple(sorted(cc_set)))
    if key in _PROGRAM_CACHE:
        return _PROGRAM_CACHE[key]
    ADT = F32R if use_f32r else F32

    nc = bacc.Bacc("TRN2", target_bir_lowering=False, num_devices=8)

    # layer-1 inputs/weights ride fp16: same PE rate, half the DMA bytes
    img = nc.dram_tensor("img", (CIN, ROWS, COLS), F16, kind="ExternalInput")
    kin = nc.dram_tensor("kin", (CIN, GRID_P, GRID_P), F16, kind="ExternalInput")
    cbias_d = nc.dram_tensor("cbias", (128, SLAB, W), F16, kind="ExternalInput")
    kbias_d = nc.dram_tensor("kbias", (128, S, S), F16, kind="ExternalInput")
    wdr = {}
    for pre in ("f", "k"):
        wdr[f"{pre}w0a"] = nc.dram_tensor(f"{pre}w0a", (128, 9, 128), F16, kind="ExternalInput")
        wdr[f"{pre}w0b"] = nc.dram_tensor(f"{pre}w0b", (128, 9, 128), F16, kind="ExternalInput")
        for l in (1, 2, 3):
            wdr[f"{pre}w{l}"] = nc.dram_tensor(f"{pre}w{l}", (128, 9, 128), ADT, kind="ExternalInput")
    fowT_d = nc.dram_tensor("fowT", (128, 128), ADT, kind="ExternalInput")
    kowT_d = nc.dram_tensor("kowT", (128, C * L), F32, kind="ExternalInput")
    mask_gb_d = nc.dram_tensor("mask_gb", (128, 10), F32, kind="ExternalInput")
    k_gb_d = nc.dram_tensor("k_gb", (128, 8), F32, kind="ExternalInput")
    kobT_d = nc.dram_tensor("kobT", (128, L), F32, kind="ExternalInput")
    onehot_g_d = nc.dram_tensor("onehot_g", (128, 32), F32, kind="ExternalInput")
    onehot_b_d = nc.dram_tensor("onehot_b", (32, 128), F32, kind="ExternalInput")
    selP_d = nc.dram_tensor("selP", (128, NCH * KSEL), F32, kind="ExternalInput")
    out_d = nc.dram_tensor("out", (KL, HALF * W), F16, kind="ExternalOutput")
    cc_d = [(nc.dram_tensor(f"ccin{l}", (32, 2), F32),
             nc.dram_tensor(f"ccout{l}", (32, 2), F32)) for l in range(5)]

    with tile.TileContext(nc) as tc, contextlib.ExitStack() as ctx:
        consts = ctx.enter_context(tc.tile_pool(name="consts", bufs=1))
        acts = ctx.enter_context(tc.tile_pool(name="acts", bufs=1))
        wpool = ctx.enter_context(tc.tile_pool(name="wpool", bufs=3))
        small = ctx.enter_context(tc.tile_pool(name="small", bufs=1))
        stage = ctx.enter_context(tc.tile_pool(name="stage", bufs=2))
        bpool = ctx.enter_context(tc.tile_pool(name="bpool", bufs=2))
        ost = ctx.enter_context(tc.tile_pool(name="ost", bufs=3))
        kpool = ctx.enter_context(tc.tile_pool(name="kpool", bufs=1))
        ps_conv = ctx.enter_context(tc.tile_pool(name="ps_conv", bufs=4, space="PSUM"))
        ps_k = ctx.enter_context(tc.tile_pool(name="ps_k", bufs=2, space="PSUM"))
        ps_small = ctx.enter_context(tc.tile_pool(name="ps_small", bufs=2, space="PSUM"))

        # ------------- layer-1 weights first on the gpsimd queue -------------
        def load_w(nm, tag, eng=None, dt=None):
            dr = wdr[nm]
            t = wpool.tile([dr.shape[0], 9, 128], dt or ADT, tag=tag, name=nm)
            (eng or nc.sync).dma_start(out=t, in_=dr[:, :, :])
            return t

        fw0 = [load_w("fw0a", "fw", eng=nc.gpsimd, dt=F16),
               load_w("fw0b", "fw", eng=nc.gpsimd, dt=F16)]

        # ------------- constants (gpsimd queue; tiny) -------------
        def load_const(dram, shape, dt=F32):
            t = consts.tile(shape, dt, tag=dram.name, name=dram.name)
            nc.gpsimd.dma_start(out=t, in_=dram[tuple(slice(None) for _ in shape)])
            return t

        onehot_g = load_const(onehot_g_d, [128, 32])
        onehot_b = load_const(onehot_b_d, [32, 128])
        mask_gb = load_const(mask_gb_d, [128, 10])
        k_gb = load_const(k_gb_d, [128, 8])
        kobT = load_const(kobT_d, [128, L])
        selP = load_const(selP_d, [128, NCH * KSEL])
        fowT = load_const(fowT_d, [128, 128], ADT)
        eps32 = consts.tile([32, 1], F32, tag="eps32", name="eps32")
        nc.vector.memset(eps32, EPS)
        ident = consts.tile([128, 128], F32, tag="ident", name="ident")
        make_identity(nc, ident)
        # fp16 identity: adds host bias maps into PSUM at 1 cycle/row
        ident_r = consts.tile([128, 128], F16, tag="ident_r", name="ident_r")
        nc.vector.tensor_copy(ident_r[:], ident[:])

        # activation slabs (ping-pong); pads zeroed via tensor_copy from an
        # F32 zero tile (memset cannot produce f32r, tensor_copy can)
        zsrc = consts.tile([128, COLS], F32, tag="zsrc", name="zsrc")
        nc.vector.memset(zsrc, 0.0)
        zcol = zsrc[:, 0:ROWS].rearrange("p (a b) -> p a b", b=1)

        X = [acts.tile([128, ROWS, COLS], ADT, tag="X0", name="X0"),
             acts.tile([128, ROWS, COLS], ADT, tag="X1", name="X1")]
        for x in X:
            nc.vector.tensor_copy(x[:, 0, :], zsrc[:, :])
            nc.vector.tensor_copy(x[:, ROWS - 1, :], zsrc[:, :])
            nc.vector.tensor_copy(x[:, :, 0:1], zcol)
            nc.vector.tensor_copy(x[:, :, COLS - 1:COLS], zcol)
        KB = [kpool.tile([128, GRID_P, GRID_P], ADT, tag="KB0", name="KB0"),
              kpool.tile([128, GRID_P, GRID_P], ADT, tag="KB1", name="KB1")]
        zcolk = zsrc[:, 0:GRID_P].rearrange("p (a b) -> p a b", b=1)
        for x in KB:
            nc.vector.tensor_copy(x[:, 0, :], zsrc[:, 0:GRID_P])
            nc.vector.tensor_copy(x[:, GRID_P - 1, :], zsrc[:, 0:GRID_P])
            nc.vector.tensor_copy(x[:, :, 0:1], zcolk)
            nc.vector.tensor_copy(x[:, :, GRID_P - 1:GRID_P], zcolk)

        # kin chunks (tags shared with later k-branch tiles to save SBUF)
        kinA = kpool.tile([128, GRID_P, GRID_P], F16, tag="sh_kow", name="sh_kow")
        kinB = kpool.tile([128, GRID_P, GRID_P], F16, tag="sh_hidT", name="sh_hidT")
        nc.gpsimd.dma_start(out=kinA, in_=kin[0:128, :, :])
        nc.gpsimd.dma_start(out=kinB, in_=kin[128:256, :, :])

        # ------------- GroupNorm (split emitters) -------------
        # stats are accumulated per-conv-tile straight from PSUM into a
        # [128, ntiles, 6] tile; gn_start aggregates + triggers the pair CC,
        # gn_finish consumes the CC result (PE work goes in between).
        def stats_tile(fam, ntiles):
            return small.tile([128, ntiles, 6], F32, tag=f"bnst{fam}",
                              name=f"bnst{fam}")

        def gn_start(stats, divisor, cc, fam):
            mv = small.tile([128, 2], F32, tag=f"mv{fam}", name=f"mv{fam}")
            nc.vector.bn_aggr(out=mv[:], in_=stats.rearrange("p a b -> p (a b)"))
            sq = small.tile([128, 2], F32, tag=f"sq{fam}", name=f"sq{fam}")
            nc.vector.tensor_mul(sq[:, 1:2], mv[:, 0:1], mv[:, 0:1])
            nc.vector.tensor_add(sq[:, 1:2], sq[:, 1:2], mv[:, 1:2])
            nc.vector.tensor_copy(sq[:, 0:1], mv[:, 0:1])

            pg = ps_small.tile([32, 2], F32, tag="pss", name="pss")
            nc.tensor.matmul(pg, onehot_g[:], sq[:], start=True, stop=True)
            g32 = small.tile([32, 2], F32, tag=f"g32{fam}", name=f"g32{fam}")
            nc.vector.tensor_copy(g32[:], pg)
            if cc is not None:
                ccin_d, ccout_d = cc
                nc.gpsimd.dma_start(out=ccin_d[:, :], in_=g32[:])
                nc.gpsimd.collective_compute(
                    "AllReduce", mybir.AluOpType.add,
                    replica_groups=[[0, 1], [2, 3], [4, 5], [6, 7]],
                    ins=[ccin_d[:, :]], outs=[ccout_d[:, :]])
            return g32

        def gn_finish(g32, divisor, gb_ap, cc, fam):
            if cc is not None:
                _, ccout_d = cc
                g32 = small.tile([32, 2], F32, tag=f"g32r{fam}", name=f"g32r{fam}")
                nc.gpsimd.dma_start(out=g32[:], in_=ccout_d[:, :])
            nc.vector.tensor_scalar_mul(g32[:], g32[:], 1.0 / divisor)
            msq = small.tile([32, 1], F32, tag=f"msq{fam}", name=f"msq{fam}")
            nc.vector.tensor_mul(msq[:], g32[:, 0:1], g32[:, 0:1])
            nc.vector.tensor_sub(g32[:, 1:2], g32[:, 1:2], msq[:])
            nc.scalar.activation(out=g32[:, 1:2], in_=g32[:, 1:2],
                                 func=mybir.ActivationFunctionType.Sqrt,
                                 bias=eps32[:], scale=1.0)
            nc.vector.reciprocal(g32[:, 1:2], g32[:, 1:2])

            pb = ps_small.tile([128, 2], F32, tag="pss", name="pss")
            nc.tensor.matmul(pb, onehot_b[:], g32[:], start=True, stop=True)
            mr = small.tile([128, 2], F32, tag=f"mr{fam}", name=f"mr{fam}")
            nc.vector.tensor_copy(mr[:], pb)

            ab = small.tile([128, 2], F32, tag=f"ab{fam}", name=f"ab{fam}")
            nc.vector.tensor_mul(ab[:, 0:1], gb_ap[:, 0:1], mr[:, 1:2])
            nc.vector.tensor_mul(ab[:, 1:2], mr[:, 0:1], ab[:, 0:1])
            nc.vector.tensor_sub(ab[:, 1:2], gb_ap[:, 1:2], ab[:, 1:2])
            return ab

        def apply_gn(dst, ab, nrows, ncols, chunks=(6, 18, 18, 18, 18, 18)):
            r = 0
            ci = 0
            while r < nrows:
                take = min(chunks[ci] if ci < len(chunks) else 18, nrows - r)
                nc.scalar.activation(
                    out=dst[:, 1 + r:1 + r + take, 1:1 + ncols],
                    in_=dst[:, 1 + r:1 + r + take, 1:1 + ncols].bitcast(F32),
                    func=mybir.ActivationFunctionType.Relu,
                    scale=ab[:, 0:1], bias=ab[:, 1:2])
                r += take
                ci += 1

        # ------------- conv emitters -------------
        def mask_conv_layer(src_fn, wts, dst, stats, bias_fn=None):
            """28 tiles of 3 rows; one bn_stats per tile straight from PSUM
            (valid rows only) so the CC can trigger as soon as the last tile
            finishes.  A host bias map is accumulated into PSUM via an f32r
            identity matmul (so stats include it)."""
            n_chunks = len(wts)
            for t in range(NT):
                if bias_fn is not None:
                    btile = bias_fn(t)
                ps = ps_conv.tile([128, 3, W], F32, tag="pc", name="pc")
                first = True
                for ci in range(n_chunks):
                    rhs_tile = src_fn(t, ci)
                    for tap in range(9):
                        dy, dx = divmod(tap, 3)
                        last = (ci == n_chunks - 1 and tap == 8
                                and bias_fn is None)
                        nc.tensor.matmul(
                            ps, wts[ci][:, tap, :],
                            rhs_tile[:, dy:dy + 3, dx:dx + W],
                            start=first, stop=last)
                        first = False
                if bias_fn is not None:
                    nc.tensor.matmul(ps, ident_r[:], btile, start=False,
                                     stop=True)
                # stats: tiles 0..25 all 3 rows, tile 26 first 2 rows
                if t < 26:
                    nc.vector.bn_stats(out=stats[:, t, :],
                                       in_=ps.rearrange("p a b -> p (a b)"))
                elif t == 26:
                    nc.vector.bn_stats(
                        out=stats[:, t, :],
                        in_=ps[:, 0:2, :].rearrange("p a b -> p (a b)"))
                nc.vector.tensor_copy(dst[:, 1 + 3 * t:4 + 3 * t, 1:1 + W], ps)

        def k_conv_layer(srcs, wts, dst, stats, bias_fn=None):
            n_chunks = len(wts)
            for t in range(4):
                if bias_fn is not None:
                    btile = bias_fn(t)
                ps = ps_k.tile([128, 10, S], F32, tag="pk", name="pk")
                first = True
                for ci in range(n_chunks):
                    for tap in range(9):
                        dy, dx = divmod(tap, 3)
                        last = (ci == n_chunks - 1 and tap == 8
                                and bias_fn is None)
                        nc.tensor.matmul(
                            ps, wts[ci][:, tap, :],
                            srcs[ci][:, 10 * t + dy:10 * t + dy + 10, dx:dx + S],
                            start=first, stop=last)
                        first = False
                if bias_fn is not None:
                    nc.tensor.matmul(ps, ident_r[:], btile, start=False,
                                     stop=True)
                nc.vector.bn_stats(out=stats[:, t, :],
                                   in_=ps.rearrange("p a b -> p (a b)"))
                nc.vector.tensor_copy(dst[:, 1 + 10 * t:11 + 10 * t, 1:1 + S],
                                      ps)

        # ------------- mask layer 1 (streamed) + k layer 1 -------------
        l1_stages = {}

        def l1_src(t, ci):
            if (t, ci) not in l1_stages:
                st = stage.tile([128, 5, COLS], F16, tag=f"st{ci}", name=f"st{ci}")
                eng = nc.sync if ci == 0 else nc.scalar
                eng.dma_start(out=st, in_=img[128 * ci:128 * ci + 128,
                                              3 * t:3 * t + 5, :])
                l1_stages[(t, ci)] = st
            return l1_stages[(t, ci)]

        def l1_bias(t):
            bt = bpool.tile([128, 3, W], F16, tag="cb", name="cb")
            nc.sync.dma_start(out=bt, in_=cbias_d[:, 3 * t:3 * t + 3, :])
            return bt

        def k1_bias(t):
            bt = bpool.tile([128, 10, S], F16, tag="cb", name="cb")
            nc.sync.dma_start(out=bt, in_=kbias_d[:, 10 * t:10 * t + 10, :])
            return bt

        # k layer-1 weights early on the gpsimd queue (sync is busy streaming)
        kw0 = [load_w("kw0a", "kw", eng=nc.gpsimd, dt=F16),
               load_w("kw0b", "kw", eng=nc.gpsimd, dt=F16)]

        st_m = stats_tile("m", 27)
        mask_conv_layer(l1_src, fw0, X[0], st_m, bias_fn=l1_bias)
        g32m = gn_start(st_m, 8.0, cc_d[0] if 0 in cc_set else None, "m")

        st_k = stats_tile("k", 4)
        k_conv_layer([kinA, kinB], kw0, KB[0], st_k, bias_fn=k1_bias)
        g32k = gn_start(st_k, 4.0, None, "k")
        abk = gn_finish(g32k, 4.0, k_gb[:, 0:2], None, "k")
        apply_gn(KB[0], abk, S, S, chunks=(40,))
        ab = gn_finish(g32m, 8.0, mask_gb[:, 0:2],
                       cc_d[0] if 0 in cc_set else None, "m")
        apply_gn(X[0], ab, SLAB, W)

        # ------------- layers 2..4 -------------
        cur = 0
        kcur = 0
        for l in (1, 2, 3):
            fwl = [load_w(f"fw{l}", "fw")]
            kwl = [load_w(f"kw{l}", "kw")]

            def src(t, ci, _cur=cur):
                return X[_cur][:, 3 * t:3 * t + 5, :]

            st_m = stats_tile("m", 27)
            mask_conv_layer(src, fwl, X[1 - cur], st_m)
            g32m = gn_start(st_m, 8.0, cc_d[l] if l in cc_set else None, "m")
            st_k = stats_tile("k", 4)
            k_conv_layer([KB[kcur]], kwl, KB[1 - kcur], st_k)
            g32k = gn_start(st_k, 4.0, None, "k")
            abk = gn_finish(g32k, 4.0, k_gb[:, 2 * l:2 * l + 2], None, "k")
            apply_gn(KB[1 - kcur], abk, S, S, chunks=(40,))
            ab = gn_finish(g32m, 8.0, mask_gb[:, 2 * l:2 * l + 2],
                           cc_d[l] if l in cc_set else None, "m")
            apply_gn(X[1 - cur], ab, SLAB, W)
            if l == 1:
                # kowT into kinA's slot (free since k layer-1); emitted here
                # so its gpsimd-queue slot sits after ccout1, long before use
                kowT = kpool.tile([128, C * L], F32, tag="sh_kow", name="sh_kow")
                nc.gpsimd.dma_start(out=kowT, in_=kowT_d[:, :])
            cur, kcur = 1 - cur, 1 - kcur

        khid = KB[kcur]   # l4 output (kcur flipped 3 times: KB[1])

        # ------------- fow conv (stats from PSUM) -------------
        mf = X[1 - cur]
        st_f = stats_tile("m", 27)
        for t in range(27):
            ps = ps_conv.tile([128, 3, W], F32, tag="pc", name="pc")
            nc.tensor.matmul(ps, fowT[:],
                             X[cur][:, 1 + 3 * t:4 + 3 * t, 1:1 + W],
                             start=True, stop=True)
            if t < 26:
                nc.vector.bn_stats(out=st_f[:, t, :],
                                   in_=ps.rearrange("p a b -> p (a b)"))
            else:
                nc.vector.bn_stats(
                    out=st_f[:, t, :],
                    in_=ps[:, 0:2, :].rearrange("p a b -> p (a b)"))
            if t % 2 == 0:
                nc.vector.tensor_copy(mf[:, 1 + 3 * t:4 + 3 * t, 1:1 + W], ps)
            else:
                nc.scalar.copy(out=mf[:, 1 + 3 * t:4 + 3 * t, 1:1 + W], in_=ps)
        g32f = gn_start(st_f, 8.0, cc_d[4] if 4 in cc_set else None, "m")

        # ------------- gather + kow (fills fow's AllReduce window) -------------
        hidc = kpool.tile([128, S, S], F32, tag="KB0", name="KB0")  # reuse KB0
        nc.vector.tensor_copy(hidc[:], khid[:, 1:1 + S, 1:1 + S].bitcast(F32))
        hidcf = hidc.rearrange("p a b -> p (a b)")
        hidT = kpool.tile([128, NCH, 128], F32, tag="sh_hidT", name="sh_hidT")
        for j in range(NCH):
            npos = min(128, NPOS - 128 * j)
            pt = ps_small.tile([128, 128], F32, tag="pss", name="pss")
            nc.tensor.transpose(pt[0:npos, :], hidcf[:, 128 * j:128 * j + npos],
                                ident)
            nc.vector.tensor_copy(hidT[0:npos, j, :], pt[0:npos, :])
        psel = ps_small.tile([128, KSEL], F32, tag="pss", name="pss")
        selPv = selP.rearrange("p (j k) -> p j k", j=NCH)
        for j in range(NCH):
            npos = min(128, NPOS - 128 * j)
            nc.tensor.matmul(psel, hidT[0:npos, j, :], selPv[0:npos, j, :],
                             start=(j == 0), stop=(j == NCH - 1))
        hsel = small.tile([128, KSEL], F32, tag="hsel", name="hsel")
        nc.vector.tensor_copy(hsel[:], psel)

        kernT = small.tile([128, KL], F32, tag="kernT", name="kernT")
        kernTv = kernT.rearrange("p (k l) -> p k l", l=L)
        for l in range(L):
            pk = ps_small.tile([128, KSEL], F32, tag="pss", name="pss")
            nc.tensor.matmul(pk, kowT[:, 128 * l:128 * l + 128], hsel[:],
                             start=True, stop=True)
            nc.vector.tensor_scalar_add(kernTv[:, :, l], pk, kobT[:, l:l + 1])
        kernTr = small.tile([128, KL], ADT, tag="kernTr", name="kernTr")
        nc.vector.tensor_copy(kernTr[:], kernT[:])

        ab = gn_finish(g32f, 8.0, mask_gb[:, 8:10],
                       cc_d[4] if 4 in cc_set else None, "m")
        apply_gn(mf, ab, HALF + 1, W)

        # ------------- einsum + fp16 output (6 conv tiles per DMA) -------------
        groups = [(0, 6), (6, 6), (12, 6), (18, 6), (24, 3)]
        for gi, (m0, mm) in enumerate(((0, 128), (128, 128), (256, 128),
                                       (384, KL - 384))):
            for t0, gn_ in groups:
                grows = sum(3 if t < 26 else 2 for t in range(t0, t0 + gn_))
                st = ost.tile([128, 18, W], F16, tag="ot", name="ot")
                for ti, t in enumerate(range(t0, t0 + gn_)):
                    rows = 3 if t < 26 else 2
                    ps = ps_conv.tile([128, 3, W], F32, tag="pc", name="pc")
                    nc.tensor.matmul(
                        ps[0:mm, 0:rows, :], kernTr[:, m0:m0 + mm],
                        mf[:, 1 + 3 * t:1 + 3 * t + rows, 1:1 + W],
                        start=True, stop=True)
                    if t % 2 == 0:
                        nc.vector.tensor_copy(
                            st[0:mm, 3 * ti:3 * ti + rows, :],
                            ps[0:mm, 0:rows, :])
                    else:
                        nc.scalar.copy(out=st[0:mm, 3 * ti:3 * ti + rows, :],
                                       in_=ps[0:mm, 0:rows, :])
                eng = nc.sync if (t0 // 6) % 2 == 0 else nc.gpsimd
                eng.dma_start(
                    out=out_d[m0:m0 + mm,
                              3 * W * t0:3 * W * t0 + grows * W],
                    in_=st[0:mm, 0:grows, :])

    nc.compile()
    _PROGRAM_CACHE[key] = nc
    return nc


# --------------------------------------------------------------------------
# host glue
# --------------------------------------------------------------------------

def _prep_inputs(inputs, use_f32r=True):
    feats = np.asarray(inputs['feats'], np.float32)
    bboxes = np.asarray(inputs['matched_bboxes'])
    idx = topk_idx(bboxes, KSEL)

    rnd = _round_f32r if use_f32r else (lambda x: np.ascontiguousarray(x, np.float32))

    xx, yy = np.meshgrid(np.linspace(-1, 1, W, dtype=np.float64),
                         np.linspace(-1, 1, H, dtype=np.float64), indexing='xy')
    coord = np.stack([xx, yy]).astype(np.float32)
    Rh = resize_weight_mat(H, S).astype(np.float32)
    Rw = resize_weight_mat(W, S).astype(np.float32)

    def conv_w_prep(w, flip):
        w = np.asarray(w, np.float32)
        if flip:
            w = w[:, :, ::-1, :]
        return np.ascontiguousarray(
            w.transpose(1, 2, 3, 0).reshape(w.shape[1], 9, w.shape[0]))

    def gb_pack(pairs):
        out = np.zeros((128, 2 * len(pairs)), np.float32)
        for i, (g, b) in enumerate(pairs):
            out[:, 2 * i] = np.asarray(g, np.float32)
            out[:, 2 * i + 1] = np.asarray(b, np.float32)
        return out

    mask_gb = gb_pack([(inputs[f'fg{l}'], inputs[f'fb{l}']) for l in range(4)]
                      + [(inputs['fog'], inputs['fob'])])
    k_gb = gb_pack([(inputs[f'kg{l}'], inputs[f'kb{l}']) for l in range(4)])

    kow = np.asarray(inputs['kow'], np.float32).reshape(C * L, C)
    kowT = np.ascontiguousarray(kow.T)
    kob = np.asarray(inputs['kob'], np.float32)
    kobT = np.ascontiguousarray(kob.reshape(L, C).T)
    fow = np.asarray(inputs['fow'], np.float32).reshape(C, C)
    fowT = rnd(np.ascontiguousarray(fow.T))

    onehot_g = np.zeros((128, 32), np.float32)
    onehot_g[np.arange(128), np.arange(128) // 4] = 1.0
    onehot_b = np.ascontiguousarray(onehot_g.T)

    per_orient = {}
    kw9c = {}
    for flip in (False, True):
        wd = {}
        for pre, key in (("f", "fw"), ("k", "kw")):
            w0 = conv_w_prep(inputs[f'{key}0'], flip)
            wd[f"{pre}w0a"] = w0[0:128].astype(np.float16)
            wd[f"{pre}w0b"] = w0[128:256].astype(np.float16)
            for l in (1, 2, 3):
                wd[f"{pre}w{l}"] = rnd(conv_w_prep(inputs[f'{key}{l}'], flip))
        # coord-channel contribution to the layer-1 conv output, as a
        # host-precomputed bias map [128, SLAB, W] (added during PSUM drain)
        coord_or = coord[:, ::-1, :] if flip else coord
        cslab = np.zeros((2, ROWS, COLS), np.float32)
        cslab[:, 1:, 1:1 + W] = coord_or[:, 0:ROWS - 1, :]
        w9c = conv_w_prep(inputs['fw0'], flip)[256:258]
        cb = np.zeros((128, SLAB, W), np.float32)
        for tap in range(9):
            dy, dx = divmod(tap, 3)
            cb += np.einsum('co,crw->orw', w9c[:, tap, :],
                            cslab[:, dy:dy + SLAB, dx:dx + W])
        wd["cbias"] = cb.astype(np.float16)
        kw9c[flip] = conv_w_prep(inputs['kw0'], flip)[256:258]
        per_orient[flip] = wd

    in_maps = []
    for c in range(8):
        b, s = c // 2, c % 2
        base = np.concatenate([feats[b], coord], 0)
        if s == 1:
            base = base[:, ::-1, :]
        img_pad = np.zeros((CINP, ROWS, COLS), np.float32)
        img_pad[:, 1:, 1:1 + W] = base[:, 0:ROWS - 1, :]

        t = np.tensordot(Rh, base, axes=(1, 1))
        kin = np.tensordot(t, Rw, axes=(2, 1)).transpose(1, 0, 2)
        kin_pad = np.zeros((CINP, GRID_P, GRID_P), np.float32)
        kin_pad[:, 1:-1, 1:-1] = kin
        # k-tower layer-1 coord contribution (per-core: depends on the
        # resized grid of this image/orientation)
        w9kc = kw9c[s == 1]
        kb_ = np.zeros((128, S, S), np.float32)
        for kk in range(9):
            dy, dx = divmod(kk, 3)
            kb_ += np.einsum('co,crw->orw', w9kc[:, kk, :],
                             kin_pad[256:258, dy:dy + S, dx:dx + S])

        selP = np.zeros((128, NCH * KSEL), np.float32)
        for k in range(KSEL):
            y_, x_ = divmod(int(idx[b, k]), S)
            if s == 1:
                y_ = S - 1 - y_
            p = y_ * S + x_
            selP[p % 128, (p // 128) * KSEL + k] = 1.0

        m = dict(per_orient[s == 1])
        m.update(
            img=np.ascontiguousarray(img_pad[0:CIN], np.float16),
            kin=np.ascontiguousarray(kin_pad[0:CIN], np.float16),
            kbias=kb_.astype(np.float16),
            fowT=fowT, kowT=kowT, mask_gb=mask_gb, k_gb=k_gb, kobT=kobT,
            onehot_g=onehot_g, onehot_b=onehot_b, selP=selP)
        in_maps.append(m)
    return in_maps


def assemble_output(results):
    out = np.zeros((B, KL, H, W), np.float32)
    for c in range(8):
        b, s = c // 2, c % 2
        pred = np.asarray(results[c]["out"], np.float32).reshape(KL, HALF, W)
        if s == 0:
            out[b, :, 0:HALF, :] = pred
        else:
            out[b, :, HALF:, :] = pred[:, ::-1, :]
    return out.reshape(B, KSEL, L, H, W)


def kernel(**inputs) -> np.ndarray:
    use_f32r = True
    nc = build_program(use_f32r=use_f32r)
    in_maps = _prep_inputs(inputs, use_f32r=use_f32r)
    res = run_bass_kernel_spmd(nc, in_maps, core_ids=list(range(8)))
    return assemble_output(res.results)
